# revision 1
# baseline (speedup 1.0000x reference)
"""Trainium2 Bass kernel for the spatial-attention module.

Reference computation (B=32, HS=512, C=256, H=W=64, A=256):
    wh     = h_dec @ W_h + b_h                      # (B, A)
    wfm    = einsum('bchw,ca->bhwa', fm, W_fm) + b_fm
    scores = einsum('bhwa,ba->bhw', wfm, wh)
    normed = softmax(scores over h*w)
    out    = einsum('bchw,bhw->bc', fm, normed)     # (B, C)

Refactor used here: scores = einsum('bchw,bc->bhw', fm, v) + const(b)
with v = einsum('ca,ba->bc', W_fm, wh); the per-sample constant
(b_fm . wh) cancels inside softmax, so b_fm is not needed at all.
This removes the (B,H,W,A) intermediate entirely and makes the kernel
memory-bound on the two passes over fm (134 MB), which stays resident
in SBUF so HBM is only read once.

Sharding: data-parallel over the batch axis, 4 samples per NeuronCore,
8 cores, no cross-core communication.
"""

import numpy as np

import concourse.bacc as bacc
import concourse.bass as bass
import concourse.tile as tile
from concourse import bass_utils, mybir
from concourse.masks import make_identity

F32 = mybir.dt.float32

N_CORES = 8
B = 32
BS = B // N_CORES  # samples per core
HS = 512
C = 256
A = 256
NPIX = 64 * 64  # 4096
CP = 128  # partition chunk
CC = C // CP  # 2 c-chunks
AC = A // CP  # 2 a-chunks
KC = HS // CP  # 4 hs-chunks
PCH = 512  # pixels per scores chunk (fp32 moving-operand max)
NJ = NPIX // PCH  # 8 chunks per sample
PIECE = 2048  # pixels per fm DMA piece
NPIECE = NPIX // PIECE  # 4 pieces per (b, cc)
SOFTMAX_SHIFT = 60.0  # compile-time softmax shift (scores stay < ~88-60)
F32R = True  # single-pass fp32r scores matmuls (2x fewer PE passes)
BF16_CTX = False  # context pass in bf16 (DVE 2x mode); scores stay f32r
F32R_DT = mybir.dt.float32r
BF16 = mybir.dt.bfloat16


def _build_program(stage=99):
    nc = bacc.Bacc("TRN2", target_bir_lowering=False, debug=False)

    h_dec_d = nc.dram_tensor("h_dec", (BS, HS), F32, kind="ExternalInput")
    fm_d = nc.dram_tensor(
        "fm", (BS, C, 64, 64), F32R_DT if F32R else F32, kind="ExternalInput"
    )
    w_fm_d = nc.dram_tensor("W_fm", (C, A), F32, kind="ExternalInput")
    w_h_d = nc.dram_tensor(
        "W_h", (HS, A), F32R_DT if F32R else F32, kind="ExternalInput"
    )
    b_h_d = nc.dram_tensor(
        "b_h", (A,), F32R_DT if F32R else F32, kind="ExternalInput"
    )
    out_d = nc.dram_tensor("out", (BS, C), F32, kind="ExternalOutput")

    with tile.TileContext(nc) as tc:
        with (
            tc.tile_pool(name="consts", bufs=1) as consts,
            tc.tile_pool(name="wpool", bufs=1) as wpool,
            tc.tile_pool(name="fmpool", bufs=1) as fmpool,
            tc.tile_pool(name="smax", bufs=4) as smax,
            tc.tile_pool(name="scratch", bufs=2) as scratch_pool,
            tc.tile_pool(name="psum", bufs=1, space="PSUM") as pp,
        ):
            # ---- weight DMAs first: ordered so each phase-0/1 stage's
            # input lands just before the stage needs it
            h_dec_sb = wpool.tile([BS, HS], F32)
            nc.sync.dma_start(out=h_dec_sb, in_=h_dec_d.ap())
            w_h_sb = wpool.tile([128, KC, A], F32R_DT if F32R else F32)
            nc.sync.dma_start(
                out=w_h_sb, in_=w_h_d.ap().rearrange("(kc kp) a -> kp kc a", kp=128)
            )
            b_h_sb = wpool.tile([1, A], F32R_DT if F32R else F32)
            nc.sync.dma_start(out=b_h_sb, in_=b_h_d.ap().rearrange("(o a) -> o a", o=1))
            w_fm_sb = wpool.tile([128, CC, A], F32)
            nc.sync.dma_start(
                out=w_fm_sb, in_=w_fm_d.ap().rearrange("(cc cp) a -> cp cc a", cp=128)
            )
            # ---- fm resident in SBUF (b-major so sample 0 lands first) ----
            # Piece layout per (b, cc): list of (pixel_offset, npix).  The
            # last sample's last group is split into PCH-sized pieces so only
            # ~2us of dependent compute remains once the HBM stream ends.
            def piece_layout(b):
                spans = [(i * PIECE, PIECE) for i in range(NPIECE - 1)]
                base = (NPIECE - 1) * PIECE
                if b == BS - 1:
                    spans += [(base + k * PCH, PCH) for k in range(PIECE // PCH)]
                else:
                    spans += [(base, PIECE)]
                return spans

            fm_v = fm_d.ap().rearrange("b (cc cp) h w -> b cc cp (h w)", cp=128)
            fm_sb = {}
            for b in range(BS):
                for pi, (off, npx) in enumerate(piece_layout(b)):
                    for cc in range(CC):
                        t = fmpool.tile(
                            [128, npx], F32R_DT if F32R else F32,
                            name=f"fm_{b}_{cc}_{pi}", tag=f"fm_{b}_{cc}_{pi}",
                        )
                        nc.sync.dma_start(out=t, in_=fm_v[b, cc, :, off : off + npx])
                        fm_sb[(b, cc, pi)] = t

            def fm_chunk(b, cc, j):
                """fm slice [128, PCH] for scores chunk j (pixels j*PCH...)."""
                lo = j * PCH
                for pi, (off, npx) in enumerate(piece_layout(b)):
                    if off <= lo < off + npx:
                        t = fm_sb[(b, cc, pi)]
                        return t[:, lo - off : lo - off + PCH]
                raise AssertionError

            # ---- constants ------------------------------------------------
            identity = consts.tile([128, 128], F32)
            make_identity(nc, identity)
            ones4_f = consts.tile([1, BS], F32)
            nc.vector.memset(ones4_f, 1.0)
            ones4 = consts.tile([1, BS], F32R_DT if F32R else F32)
            nc.scalar.copy(ones4, ones4_f)
            ones_row = consts.tile([1, 128], F32)
            nc.vector.memset(ones_row, 1.0)

            # ---- phase 0: whT[a,b] = (h_dec @ W_h + b_h).T ----------------
            hdT_ps = pp.tile([128, KC, BS], F32, tag="mm", bufs=2)
            for kc in range(KC):
                nc.tensor.transpose(
                    hdT_ps[:, kc, :],
                    h_dec_sb[:, kc * 128 : (kc + 1) * 128],
                    identity[0:BS, 0:BS],
                )
            hdT_sb = wpool.tile([128, KC, BS], F32R_DT if F32R else F32)
            nc.scalar.copy(hdT_sb, hdT_ps)

            whT_sb = wpool.tile([128, AC, BS], F32R_DT if F32R else F32)
            for ac in range(AC):
                whT_ps = pp.tile([128, BS], F32, tag="mm", bufs=2)
                for kc in range(KC):
                    nc.tensor.matmul(
                        whT_ps,
                        w_h_sb[:, kc, ac * 128 : (ac + 1) * 128],
                        hdT_sb[:, kc, :],
                        start=(kc == 0),
                        stop=False,
                    )
                nc.tensor.matmul(
                    whT_ps,
                    b_h_sb[0:1, ac * 128 : (ac + 1) * 128],
                    ones4,
                    start=False,
                    stop=True,
                )
                nc.scalar.copy(whT_sb[:, ac, :], whT_ps)

            # ---- phase 1: vT[c,b] = sum_a W_fm[c,a] * wh[b,a] -------------
            wfmT_sb = wpool.tile([128, AC, CC, 128], F32R_DT if F32R else F32)
            for cc in range(CC):
                for ac in range(AC):
                    wfmT_ps = pp.tile([128, 128], F32, tag="mm", bufs=2)
                    nc.tensor.transpose(
                        wfmT_ps,
                        w_fm_sb[:, cc, ac * 128 : (ac + 1) * 128],
                        identity,
                    )
                    nc.scalar.copy(wfmT_sb[:, ac, cc, :], wfmT_ps)

            vT_sb = wpool.tile([128, CC, BS], F32R_DT if F32R else F32)
            for cc in range(CC):
                vT_ps = pp.tile([128, BS], F32, tag="mm", bufs=2)
                for ac in range(AC):
                    nc.tensor.matmul(
                        vT_ps,
                        wfmT_sb[:, ac, cc, :],
                        whT_sb[:, ac, :],
                        start=(ac == 0),
                        stop=(ac == AC - 1),
                    )
                nc.scalar.copy(vT_sb[:, cc, :], vT_ps)

            # ---- consts for softmax / context ------------------------------
            negshift = consts.tile([128, 1], F32)
            nc.vector.memset(negshift, -SOFTMAX_SHIFT)
            one_col = consts.tile([128, 1], F32)
            nc.vector.memset(one_col, 1.0)

            # ---- main per-sample pipeline ---------------------------------
            # scores come out of PE replicated on all 128 partitions (vrep
            # stationary), so exp output is directly the broadcast operand
            # the context multiply needs.  softmax shift-invariance lets us
            # use a compile-time bias of -SOFTMAX_SHIFT instead of the data
            # max (scores stay well inside fp32 exp range).
            ctx_sb = wpool.tile([128, BS, CC], F32)
            out_v = out_d.ap().rearrange("b (cc cp) -> cp b cc", cp=128)
            if stage < 1:
                nc.vector.memset(ctx_sb, 0.0)
                nc.sync.dma_start(out=out_v, in_=ctx_sb)
            for b in range(BS) if stage >= 1 else []:
                zparts = smax.tile([128, NJ], F32, tag="zparts", bufs=2)
                parts = smax.tile([128, CC, NJ // 2], F32, tag="parts", bufs=2)
                tailparts = smax.tile(
                    [128, CC, PIECE // PCH], F32, tag="tailparts", bufs=1
                )
                # sample 0 uses half-size groups so the first DVE pass can
                # start as early as possible
                if b == 0:
                    group_chunks = [2] * (NJ // 2)
                else:
                    group_chunks = [PIECE // PCH] * NPIECE
                j0 = 0
                for g, gch in enumerate(group_chunks):
                    last_group = b == BS - 1 and g == len(group_chunks) - 1
                    e_big = smax.tile([128, PIECE], F32, tag="e_big", bufs=3)
                    for h in range(gch):
                        j = j0 + h
                        sc_ps = pp.tile([128, PCH], F32, tag="scores", bufs=6)
                        for cc in range(CC):
                            nc.tensor.matmul(
                                sc_ps,
                                vT_sb[:, cc, b : b + 1].to_broadcast((128, 128)),
                                fm_chunk(b, cc, j),
                                start=(cc == 0),
                                stop=(cc == CC - 1),
                            )
                        nc.scalar.activation(
                            e_big[:, h * PCH : (h + 1) * PCH], sc_ps,
                            mybir.ActivationFunctionType.Exp,
                            bias=negshift, scale=1.0,
                            accum_out=zparts[:, j : j + 1],
                        )
                        if last_group:
                            # tail chunks: STT right behind each exp so almost
                            # nothing is left once the HBM stream ends
                            for cc in range(CC):
                                scr = scratch_pool.tile(
                                    [128, PCH], F32, tag="scr_tail", bufs=2
                                )
                                nc.vector.scalar_tensor_tensor(
                                    out=scr,
                                    in0=fm_chunk(b, cc, j).bitcast(F32),
                                    scalar=one_col,
                                    in1=e_big[:, h * PCH : (h + 1) * PCH],
                                    op0=mybir.AluOpType.mult,
                                    op1=mybir.AluOpType.mult,
                                    accum_out=tailparts[:, cc, h : h + 1],
                                )
                    if not last_group:
                        # context partials: fused (fm * e) multiply + pixel
                        # sum in one DVE pass over the whole group
                        npx = gch * PCH
                        pi = (j0 * PCH) // PIECE
                        off = j0 * PCH - pi * PIECE
                        for cc in range(CC):
                            scr = scratch_pool.tile([128, PIECE], F32, tag="scr")
                            nc.vector.scalar_tensor_tensor(
                                out=scr[:, :npx],
                                in0=fm_sb[(b, cc, pi)].bitcast(F32)[
                                    :, off : off + npx
                                ],
                                scalar=one_col,
                                in1=e_big[:, :npx],
                                op0=mybir.AluOpType.mult,
                                op1=mybir.AluOpType.mult,
                                accum_out=parts[:, cc, g : g + 1],
                            )
                    j0 += gch

                # Z (replicated on all partitions) and final scale by 1/Z
                z_rep = smax.tile([128, 1], F32, tag="z")
                nc.vector.tensor_reduce(
                    z_rep, zparts, axis=mybir.AxisListType.X, op=mybir.AluOpType.add
                )
                rz_rep = smax.tile([128, 1], F32, tag="rz")
                nc.vector.reciprocal(rz_rep, z_rep)
                for cc in range(CC):
                    pr = smax.tile([128, 1], F32, tag="pr")
                    ngr = len(group_chunks) - (1 if b == BS - 1 else 0)
                    nc.vector.tensor_reduce(
                        pr,
                        parts[:, cc, :ngr],
                        axis=mybir.AxisListType.X,
                        op=mybir.AluOpType.add,
                    )
                    if b == BS - 1:
                        prt = smax.tile([128, 1], F32, tag="prt")
                        nc.vector.tensor_reduce(
                            prt, tailparts[:, cc, :], axis=mybir.AxisListType.X,
                            op=mybir.AluOpType.add,
                        )
                        nc.vector.tensor_add(pr, pr, prt)
                    nc.scalar.mul(ctx_sb[:, b, cc : cc + 1], pr, rz_rep)
                if b == BS - 1:
                    nc.sync.dma_start(out=out_v, in_=ctx_sb)

    nc.compile()
    return nc


_NC_CACHE = None


def _get_program():
    global _NC_CACHE
    if _NC_CACHE is None:
        _NC_CACHE = _build_program()
    return _NC_CACHE


def kernel(**inputs):
    h_dec = np.ascontiguousarray(np.asarray(inputs["h_dec"], dtype=np.float32))
    fm = np.ascontiguousarray(np.asarray(inputs["fm"], dtype=np.float32))
    w_fm = np.ascontiguousarray(np.asarray(inputs["W_fm"], dtype=np.float32))
    w_h = np.ascontiguousarray(np.asarray(inputs["W_h"], dtype=np.float32))
    b_h = np.ascontiguousarray(np.asarray(inputs["b_h"], dtype=np.float32))

    nc = _get_program()
    in_maps = []
    for c in range(N_CORES):
        sl = slice(c * BS, (c + 1) * BS)
        in_maps.append(
            {
                "h_dec": np.ascontiguousarray(h_dec[sl]),
                "fm": np.ascontiguousarray(fm[sl]),
                "W_fm": w_fm,
                "W_h": w_h,
                "b_h": b_h,
            }
        )
    res = bass_utils.run_bass_kernel_spmd(nc, in_maps, core_ids=list(range(N_CORES)))
    return np.concatenate([r["out"] for r in res.results], axis=0)



# revision 2
# speedup vs baseline: 1.1689x; 1.1689x over previous
"""Trainium2 Bass kernel for the spatial-attention module.

Reference computation (B=32, HS=512, C=256, H=W=64, A=256):
    wh     = h_dec @ W_h + b_h                      # (B, A)
    wfm    = einsum('bchw,ca->bhwa', fm, W_fm) + b_fm
    scores = einsum('bhwa,ba->bhw', wfm, wh)
    normed = softmax(scores over h*w)
    out    = einsum('bchw,bhw->bc', fm, normed)     # (B, C)

Refactor: scores = einsum('bchw,bc->bhw', fm, v) with
v = einsum('ca,ba->bc', W_fm, wh); the per-sample constant (b_fm . wh)
cancels inside softmax, so b_fm is never needed.  This removes the
(B,H,W,A) intermediate and makes the kernel memory-bound on a single
HBM read of fm, which stays resident in SBUF.

Precision plan (rel-err budget 2e-2, achieves ~4e-3):
  - fm is converted to fp16 on the host: halves HBM traffic (8.4 MB/core)
    and doubles both PE moving-operand rate and DVE throughput.
    fp16 (11-bit mantissa) keeps the softmax scores accurate; bf16 does
    not (scores ~N(0,16), a 0.4% fm error -> ~6% softmax weight error).
  - v is cast to fp16 (scores matmul stationary).
  - exp() output e is bf16: e spans ~e^24, which overflows fp16 but not
    bf16; fp16 x bf16 inputs still satisfy the DVE 2x_1P packing rule
    (each tensor operand 2-byte, step 1, 4B-aligned).
  - All accumulations (PSUM scores, softmax Z, context partials) fp32.

Sharding: data-parallel over batch, 4 samples per NeuronCore, 8 cores,
no cross-core communication.
"""

import numpy as np

import concourse.bacc as bacc
import concourse.bass as bass
import concourse.tile as tile
from concourse import bass_utils, mybir
from concourse.masks import make_identity

F32 = mybir.dt.float32
F32R = mybir.dt.float32r
F16 = mybir.dt.float16
BF16 = mybir.dt.bfloat16

N_CORES = 8
B = 32
BS = B // N_CORES  # samples per core
HS = 512
C = 256
A = 256
NPIX = 64 * 64  # 4096
CP = 128
CC = C // CP  # 2 channel chunks
AC = A // CP
KC = HS // CP
PCH = 512  # pixels per matmul chunk (one PSUM bank)
GRP = 1024  # pixels per exp/STT group (2 PSUM banks)
NGMAX = 5
SOFTMAX_SHIFT = 60.0


def _piece_layout(b):
    """fm DMA piece spans (pixel_offset, npix) for sample b.

    ~1MB pieces for DMA efficiency; the first sample starts with halves
    so compute can begin earlier, the last sample tapers off so only a
    short dependent-compute tail remains after the HBM stream ends.
    """
    if b == 0:
        return [(0, 1024), (1024, 1024), (2048, 2048)]
    if b == BS - 1:
        return [(0, 2048), (2048, 1024), (3072, 512), (3584, 512)]
    return [(0, 2048), (2048, 2048)]


def _group_layout(b):
    """exp/STT groups (pixel_offset, npix, piece_idx); each inside one piece."""
    groups = []
    for pi, (off, npx) in enumerate(_piece_layout(b)):
        o = 0
        while o < npx:
            n = min(GRP, npx - o)
            groups.append((off + o, n, pi))
            o += n
    return groups


def _build_program():
    nc = bacc.Bacc("TRN2", target_bir_lowering=False, debug=False)

    h_dec_d = nc.dram_tensor("h_dec", (BS, HS), F32, kind="ExternalInput")
    fm_d = nc.dram_tensor("fm", (BS, C, 64, 64), F16, kind="ExternalInput")
    w_fm_d = nc.dram_tensor("W_fm", (C, A), F32, kind="ExternalInput")
    w_h_d = nc.dram_tensor("W_h", (HS, A), F32R, kind="ExternalInput")
    b_h_d = nc.dram_tensor("b_h", (A,), F32R, kind="ExternalInput")
    out_d = nc.dram_tensor("out", (BS, C), F32, kind="ExternalOutput")

    with tile.TileContext(nc) as tc:
        with (
            tc.tile_pool(name="consts", bufs=1) as consts,
            tc.tile_pool(name="wpool", bufs=1) as wpool,
            tc.tile_pool(name="fmpool", bufs=1) as fmpool,
            tc.tile_pool(name="smax", bufs=4) as smax,
            tc.tile_pool(name="scratch", bufs=2) as scratch_pool,
            tc.tile_pool(name="psum", bufs=1, space="PSUM") as pp,
        ):
            # ---- weight DMAs first (0.77 MB lead-in), then the fm stream
            h_dec_sb = wpool.tile([BS, HS], F32)
            nc.sync.dma_start(out=h_dec_sb, in_=h_dec_d.ap())
            w_h_sb = wpool.tile([128, KC, A], F32R)
            nc.sync.dma_start(
                out=w_h_sb, in_=w_h_d.ap().rearrange("(kc kp) a -> kp kc a", kp=128)
            )
            b_h_sb = wpool.tile([1, A], F32R)
            nc.sync.dma_start(out=b_h_sb, in_=b_h_d.ap().rearrange("(o a) -> o a", o=1))
            w_fm_sb = wpool.tile([128, CC, A], F32)
            nc.sync.dma_start(
                out=w_fm_sb, in_=w_fm_d.ap().rearrange("(cc cp) a -> cp cc a", cp=128)
            )

            # ---- fm resident in SBUF, fp16, [cp, cc, pix] per piece ----
            fm_v = fm_d.ap().rearrange("b (cc cp) h w -> b cp cc (h w)", cp=128)
            fm_sb = {}
            for b in range(BS):
                for pi, (off, npx) in enumerate(_piece_layout(b)):
                    t = fmpool.tile(
                        [128, CC, npx], F16,
                        name=f"fm_{b}_{pi}", tag=f"fm_{b}_{pi}",
                    )
                    nc.sync.dma_start(out=t, in_=fm_v[b, :, :, off : off + npx])
                    fm_sb[(b, pi)] = t

            # ---- constants ------------------------------------------------
            identity = consts.tile([128, 128], F32)
            make_identity(nc, identity)
            ones4_f = consts.tile([1, BS], F32)
            nc.vector.memset(ones4_f, 1.0)
            ones4 = consts.tile([1, BS], F32R)
            nc.scalar.copy(ones4, ones4_f)
            negshift = consts.tile([128, 1], F32)
            nc.vector.memset(negshift, -SOFTMAX_SHIFT)
            one_col = consts.tile([128, 1], F32)
            nc.vector.memset(one_col, 1.0)

            # ---- phase 0: whT[a,b] = (h_dec @ W_h + b_h).T ----------------
            hdT_ps = pp.tile([128, KC, BS], F32, tag="mm", bufs=2)
            for kc in range(KC):
                nc.tensor.transpose(
                    hdT_ps[:, kc, :],
                    h_dec_sb[:, kc * 128 : (kc + 1) * 128],
                    identity[0:BS, 0:BS],
                )
            hdT_sb = wpool.tile([128, KC, BS], F32R)
            nc.scalar.copy(hdT_sb, hdT_ps)

            whT_sb = wpool.tile([128, AC, BS], F32R)
            for ac in range(AC):
                whT_ps = pp.tile([128, BS], F32, tag="mm", bufs=2)
                for kc in range(KC):
                    nc.tensor.matmul(
                        whT_ps,
                        w_h_sb[:, kc, ac * 128 : (ac + 1) * 128],
                        hdT_sb[:, kc, :],
                        start=(kc == 0),
                        stop=False,
                    )
                nc.tensor.matmul(
                    whT_ps,
                    b_h_sb[0:1, ac * 128 : (ac + 1) * 128],
                    ones4,
                    start=False,
                    stop=True,
                )
                nc.scalar.copy(whT_sb[:, ac, :], whT_ps)

            # ---- phase 1: vT[c,b] = sum_a W_fm[c,a] * wh[b,a], cast fp16 --
            wfmT_sb = wpool.tile([128, AC, CC, 128], F32R)
            for cc in range(CC):
                for ac in range(AC):
                    wfmT_ps = pp.tile([128, 128], F32, tag="mm", bufs=2)
                    nc.tensor.transpose(
                        wfmT_ps,
                        w_fm_sb[:, cc, ac * 128 : (ac + 1) * 128],
                        identity,
                    )
                    nc.scalar.copy(wfmT_sb[:, ac, cc, :], wfmT_ps)

            vT_sb = wpool.tile([128, CC, BS], F16)
            for cc in range(CC):
                vT_ps = pp.tile([128, BS], F32, tag="mm", bufs=2)
                for ac in range(AC):
                    nc.tensor.matmul(
                        vT_ps,
                        wfmT_sb[:, ac, cc, :],
                        whT_sb[:, ac, :],
                        start=(ac == 0),
                        stop=(ac == AC - 1),
                    )
                nc.scalar.copy(vT_sb[:, cc, :], vT_ps)

            # ---- main per-sample pipeline ---------------------------------
            # scores come out of PE replicated on all 128 partitions (vT
            # broadcast stationary), so the exp output is directly the
            # broadcast operand the context multiply needs.  softmax
            # shift-invariance lets a compile-time -SOFTMAX_SHIFT bias
            # replace the data max (fp32 PSUM scores; bf16 e is range-safe).
            ctx_sb = wpool.tile([128, BS, CC], F32)
            out_v = out_d.ap().rearrange("b (cc cp) -> cp b cc", cp=128)
            for b in range(BS):
                groups = _group_layout(b)
                ng = len(groups)
                zparts = smax.tile([128, NGMAX], F32, tag="zparts", bufs=2)
                parts = smax.tile([128, CC, NGMAX], F32, tag="parts", bufs=2)
                for g, (goff, gnpx, pi) in enumerate(groups):
                    poff = _piece_layout(b)[pi][0]
                    lo = goff - poff
                    sc_ps = pp.tile([128, GRP], F32, tag="scores", bufs=3)
                    for h in range((gnpx + PCH - 1) // PCH):
                        co = h * PCH
                        cn = min(PCH, gnpx - co)
                        for cc in range(CC):
                            nc.tensor.matmul(
                                sc_ps[:, co : co + cn],
                                vT_sb[:, cc, b : b + 1].to_broadcast((128, 128)),
                                fm_sb[(b, pi)][:, cc, lo + co : lo + co + cn],
                                start=(cc == 0),
                                stop=(cc == CC - 1),
                            )
                    e_big = smax.tile([128, GRP], BF16, tag="e", bufs=3)
                    nc.scalar.activation(
                        e_big[:, :gnpx], sc_ps[:, :gnpx],
                        mybir.ActivationFunctionType.Exp,
                        bias=negshift, scale=1.0,
                        accum_out=zparts[:, g : g + 1],
                    )
                    for cc in range(CC):
                        scr = scratch_pool.tile([128, GRP], F16, tag="scr", bufs=3)
                        nc.vector.scalar_tensor_tensor(
                            out=scr[:, :gnpx],
                            in0=fm_sb[(b, pi)][:, cc, lo : lo + gnpx],
                            scalar=one_col,
                            in1=e_big[:, :gnpx],
                            op0=mybir.AluOpType.mult,
                            op1=mybir.AluOpType.mult,
                            accum_out=parts[:, cc, g : g + 1],
                        )

                # Z (replicated on all partitions) and final scale by 1/Z
                z_rep = smax.tile([128, 1], F32, tag="z")
                nc.vector.tensor_reduce(
                    z_rep, zparts[:, :ng], axis=mybir.AxisListType.X,
                    op=mybir.AluOpType.add,
                )
                rz_rep = smax.tile([128, 1], F32, tag="rz")
                nc.vector.reciprocal(rz_rep, z_rep)
                for cc in range(CC):
                    pr = smax.tile([128, 1], F32, tag="pr")
                    nc.vector.tensor_reduce(
                        pr, parts[:, cc, :ng], axis=mybir.AxisListType.X,
                        op=mybir.AluOpType.add,
                    )
                    nc.scalar.mul(ctx_sb[:, b, cc : cc + 1], pr, rz_rep)
                # per-sample output DMA: samples 0..BS-2 flush during the
                # stream; only the last sample's 1KB write is in the tail
                nc.sync.dma_start(
                    out=out_v[:, b : b + 1, :], in_=ctx_sb[:, b : b + 1, :]
                )

    nc.compile()
    return nc


_NC_CACHE = None


def _get_program():
    global _NC_CACHE
    if _NC_CACHE is None:
        _NC_CACHE = _build_program()
    return _NC_CACHE


def kernel(**inputs):
    h_dec = np.ascontiguousarray(np.asarray(inputs["h_dec"], dtype=np.float32))
    fm16 = np.asarray(inputs["fm"], dtype=np.float32).astype(np.float16)
    w_fm = np.ascontiguousarray(np.asarray(inputs["W_fm"], dtype=np.float32))
    w_h = np.ascontiguousarray(np.asarray(inputs["W_h"], dtype=np.float32))
    b_h = np.ascontiguousarray(np.asarray(inputs["b_h"], dtype=np.float32))

    nc = _get_program()
    in_maps = []
    for c in range(N_CORES):
        sl = slice(c * BS, (c + 1) * BS)
        in_maps.append(
            {
                "h_dec": np.ascontiguousarray(h_dec[sl]),
                "fm": np.ascontiguousarray(fm16[sl]),
                "W_fm": w_fm,
                "W_h": w_h,
                "b_h": b_h,
            }
        )
    res = bass_utils.run_bass_kernel_spmd(nc, in_maps, core_ids=list(range(N_CORES)))
    return np.concatenate([r["out"] for r in res.results], axis=0)
